# revision 15
# baseline (speedup 1.0000x reference)
"""MoREGPT Trainium2 kernel: embedding gather -> MoE recurrent scan -> LN -> tied head.

Self-contained: hardcodes shapes/sharding. 8 NeuronCores SPMD, no collectives:
- scan replicated on all cores (cheap, avoids broadcasting H)
- tied head sharded over vocab (6400 padded cols/core), host concatenates.
"""

import numpy as np

import concourse.bass as bass
import concourse.tile as tile
from concourse import bacc, mybir
from concourse.bass import ts
from concourse.bass_utils import run_bass_kernel_spmd
from concourse.masks import make_identity

P = 128
V = 50257
D = 768
E = 4
R = 32
B = 4
T = 512
N = B * T          # 2048 tokens, order n = t*B + b
NK = D // P        # 6 k-chunks of d
NC = N // P        # 16 token chunks of 128
NJ = N // 512      # 4 token chunks of 512
VSH = 6400         # padded vocab shard per core (8*6400 = 51200 >= V)
NV = VSH // P      # 50 vocab chunks of 128
LN_EPS = 1e-5
N_CORES = 8

F32 = mybir.dt.float32
F32R = mybir.dt.float32r
I32 = mybir.dt.int32
ALU = mybir.AluOpType
ACT = mybir.ActivationFunctionType
AX = mybir.AxisListType


def _sig(e):
    return (e + 1) % 4


def _emit(tc, io):
    nc = tc.nc
    w_emb = io["w_emb"]
    out = io["out"]

    _cp_cm = tc.tile_pool(name="consts", bufs=1)
    _pp_cm = tc.tile_pool(name="persist", bufs=1)
    cp = _cp_cm.__enter__()
    pp = _pp_cm.__enter__()
    dbg = io.get("dbg")

    # ---- constants / small inputs to SBUF ----
    idn = cp.tile([P, P], F32, name="idn")
    make_identity(nc, idn[:])
    idx_t = cp.tile([P, NC], I32, name="idx")
    nc.sync.dma_start(idx_t[:], io["idxs"][:])
    gt_t = cp.tile([P, NK, E], F32, name="gt")
    nc.sync.dma_start(gt_t[:], io["gt"].rearrange("(k p) e -> p k e", p=P))
    win_t = cp.tile([P, NK, E * R], F32, name="win")
    nc.sync.dma_start(win_t[:], io["win"].rearrange("(k p) c -> p k c", p=P))
    wbd_t = cp.tile([P, P], F32, name="wbd")
    nc.sync.dma_start(wbd_t[:], io["wbd"][:])
    wout_t = cp.tile([P, D], F32, name="wout")
    nc.sync.dma_start(wout_t[:], io["wout"][:])
    expt_t = cp.tile([E, P], F32, name="expt")
    nc.sync.dma_start(expt_t[:], io["expt"][:])
    gamma_t = cp.tile([1, D], F32, name="gamma")
    nc.sync.dma_start(gamma_t[:], io["gamma"][:])
    beta_t = cp.tile([P, NK], F32, name="beta")
    nc.sync.dma_start(beta_t[:], io["beta"][:])
    ones_col = cp.tile([P, 1], F32, name="ones_col")
    nc.vector.memset(ones_col[:], 1.0)
    ones_row = cp.tile([1, P], F32, name="ones_row")
    nc.vector.memset(ones_row[:], 1.0)
    eps_t = cp.tile([1, 1], F32, name="eps")
    nc.vector.memset(eps_t[:], LN_EPS)

    # ---- persistent big tiles ----
    xtt = pp.tile([P, NK, N], F32, name="xtt")      # X^T  (d on partitions)
    xin = pp.tile([P, N], F32, name="xin")          # Xin^T (sigma-permuted er rows)
    mask = pp.tile([P, N], F32, name="mask")        # winner mask (er rows)
    ct = pp.tile([P, N], F32, name="ct")            # tanh candidates -> masked C^T
    yt = pp.tile([P, NK, N], F32R, name="yt")        # Y^T -> H^T in place
    oht = pp.tile([E, N], F32, name="oht")          # onehot^T
    mean_row = pp.tile([1, N], F32, name="mean_row")
    r_row = pp.tile([1, N], F32, name="r_row")
    s_tile = pp.tile([P, B], F32, name="s_tile")    # recurrent state (er x batch)
    nc.vector.memset(s_tile[:], 0.0)

    # ================= phase 1+2: gather embeddings, transpose to X^T ========
    with tc.tile_pool(name="xg", bufs=1) as xg, \
         tc.tile_pool(name="tp_ps", bufs=4, space="PSUM") as tps:
        x = xg.tile([P, NC, D], F32, name="x")
        for c in range(NC):
            nc.gpsimd.indirect_dma_start(
                out=x[:, c, :],
                out_offset=None,
                in_=w_emb[:],
                in_offset=bass.IndirectOffsetOnAxis(ap=idx_t[:, c : c + 1], axis=0),
            )
        for c in range(NC):
            for k in range(NK):
                pt = tps.tile([P, P], F32, name="pt")
                nc.tensor.transpose(pt[:], x[:, c, ts(k, P)], idn[:])
                if (c * NK + k) % 2 == 0:
                    nc.vector.tensor_copy(xtt[:, k, ts(c, P)], pt[:])
                else:
                    nc.scalar.copy(xtt[:, k, ts(c, P)], pt[:])

    # ================= phase 3: routing scores + onehot (true fp32) ==========
    with tc.tile_pool(name="sc_ps", bufs=2, space="PSUM") as scp, \
         tc.tile_pool(name="oh_ps", bufs=2, space="PSUM") as ohp, \
         tc.tile_pool(name="sc_tmp", bufs=2) as sct:
        for c in range(NC):
            ps = scp.tile([P, E], F32, name="ps")
            for k in range(NK):
                nc.tensor.matmul(ps[:], lhsT=xtt[:, k, ts(c, P)], rhs=gt_t[:, k, :],
                                 start=(k == 0), stop=(k == NK - 1))
            mx = sct.tile([P, 1], F32, name="mx")
            nc.vector.tensor_reduce(mx[:], ps[:], axis=AX.X, op=ALU.max)
            oh = sct.tile([P, E], F32, name="oh")
            nc.vector.tensor_scalar(oh[:], ps[:], mx[:, 0:1], None, ALU.is_ge)
            po = ohp.tile([E, P], F32, name="po")
            nc.tensor.transpose(po[:], oh[:], idn[:])
            nc.vector.tensor_copy(oht[:, ts(c, P)], po[:])

    # ================= phase 4: expand mask to er rows =======================
    with tc.tile_pool(name="mk_ps", bufs=2, space="PSUM") as mkp:
        for j in range(NJ):
            pm = mkp.tile([P, 512], F32, name="pm")
            nc.tensor.matmul(pm[:], lhsT=expt_t[:], rhs=oht[:, ts(j, 512)],
                             start=True, stop=True)
            nc.scalar.copy(mask[:, ts(j, 512)], pm[:])

    # ================= phase 5: Xin^T = Win_perm^T @ X^T (fp32r) =============
    with tc.tile_pool(name="xi_ps", bufs=2, space="PSUM") as xip:
        for j in range(NJ):
            px = xip.tile([P, 512], F32, name="px")
            for k in range(NK):
                nc.tensor.matmul(px[:], lhsT=win_t[:, k, :],
                                 rhs=xtt[:, k, ts(j, 512)],
                                 start=(k == 0), stop=(k == NK - 1))
            nc.vector.tensor_copy(xin[:, ts(j, 512)], px[:])

    # ================= phase 6: the scan =====================================
    with tc.tile_pool(name="z_ps", bufs=2, space="PSUM") as zp:
        for t in range(T):
            z = zp.tile([P, B], F32, name="z")
            nc.tensor.matmul(z[:], lhsT=idn[:], rhs=xin[:, ts(t, B)],
                             start=True, stop=False)
            nc.tensor.matmul(z[:], lhsT=wbd_t[:], rhs=s_tile[:],
                             start=False, stop=True)
            nc.scalar.activation(ct[:, ts(t, B)], z[:], ACT.Tanh)
            nc.vector.copy_predicated(s_tile[:], mask[:, ts(t, B)].bitcast(I32),
                                      ct[:, ts(t, B)])

    # ================= phase 7: mask candidates ==============================
    for j in range(NJ):
        nc.vector.tensor_tensor(ct[:, ts(j, 512)], ct[:, ts(j, 512)],
                                mask[:, ts(j, 512)], ALU.mult)

    # ================= phase 8: Y^T, LN stats, normalize =====================
    with tc.tile_pool(name="y_ps", bufs=2, space="PSUM") as yp:
        for k in range(NK):
            for j in range(NJ):
                py = yp.tile([P, 512], F32, name="py")
                nc.tensor.matmul(py[:], lhsT=wout_t[:, ts(k, P)],
                                 rhs=ct[:, ts(j, 512)],
                                 start=True, stop=True)
                nc.vector.tensor_add(yt[:, k, ts(j, 512)], py[:],
                                     xtt[:, k, ts(j, 512)])

    with tc.tile_pool(name="s_ps", bufs=2, space="PSUM") as sp, \
         tc.tile_pool(name="q_ps", bufs=2, space="PSUM") as qp, \
         tc.tile_pool(name="sq_sb", bufs=2) as sqp, \
         tc.tile_pool(name="st_sb", bufs=2) as stp:
        for j in range(NJ):
            pS = sp.tile([1, 512], F32, name="pS")
            pQ = qp.tile([1, 512], F32, name="pQ")
            for k in range(NK):
                sq = sqp.tile([P, 512], F32, name="sq")
                nc.scalar.activation(sq[:], yt[:, k, ts(j, 512)], ACT.Square)
                nc.tensor.matmul(pS[:], lhsT=ones_col[:],
                                 rhs=yt[:, k, ts(j, 512)].bitcast(F32),
                                 start=(k == 0), stop=(k == NK - 1))
                nc.tensor.matmul(pQ[:], lhsT=ones_col[:],
                                 rhs=sq[:],
                                 start=(k == 0), stop=(k == NK - 1))
            nc.vector.tensor_scalar_mul(mean_row[:, ts(j, 512)], pS[:], 1.0 / D)
            tq = stp.tile([1, 512], F32, name="tq")
            nc.vector.tensor_scalar_mul(tq[:], pQ[:], 1.0 / D)
            m2 = stp.tile([1, 512], F32, name="m2")
            nc.vector.tensor_tensor(m2[:], mean_row[:, ts(j, 512)],
                                    mean_row[:, ts(j, 512)], ALU.mult)
            nc.vector.tensor_tensor(tq[:], tq[:], m2[:], ALU.subtract)
            st = stp.tile([1, 512], F32, name="st")
            nc.scalar.activation(st[:], tq[:], ACT.Sqrt, bias=eps_t[:, 0:1])
            nc.vector.reciprocal(r_row[:, ts(j, 512)], st[:])

    with tc.tile_pool(name="mb_ps", bufs=2, space="PSUM") as mbp, \
         tc.tile_pool(name="rg_ps", bufs=2, space="PSUM") as rgp, \
         tc.tile_pool(name="ap_sb", bufs=2) as app:
        for j in range(NJ):
            pmb = mbp.tile([P, 512], F32, name="pmb")
            nc.tensor.matmul(pmb[:], lhsT=ones_row[:],
                             rhs=mean_row[:, ts(j, 512)],
                             start=True, stop=True)
            for k in range(NK):
                prg = rgp.tile([P, 512], F32, name="prg")
                nc.tensor.matmul(prg[:], lhsT=gamma_t[:, ts(k, P)],
                                 rhs=r_row[:, ts(j, 512)],
                                 start=True, stop=True)
                tl = app.tile([P, 512], F32, name="tl")
                nc.vector.tensor_tensor(tl[:], yt[:, k, ts(j, 512)], pmb[:],
                                        ALU.subtract)
                nc.vector.tensor_tensor(yt[:, k, ts(j, 512)], tl[:], prg[:],
                                        ALU.mult)

    if dbg is not None:
        for k in range(NK):
            nc.sync.dma_start(dbg["xtt"][ts(k, P), :], xtt[:, k, :])
            nc.sync.dma_start(dbg["yt"][ts(k, P), :], yt[:, k, :])
        nc.sync.dma_start(dbg["oht"][:], oht[:])
        nc.sync.dma_start(dbg["mask"][:], mask[:])
        nc.sync.dma_start(dbg["xin"][:], xin[:])
        nc.sync.dma_start(dbg["ct"][:], ct[:])
        nc.sync.dma_start(dbg["mr"][0:1, :], mean_row[:])
        nc.sync.dma_start(dbg["mr"][1:2, :], r_row[:])

    # ================= phase 9: tied head (vocab shard, fp32r) ===============
    wth_r = io["wth"].rearrange("(k p) v -> p k v", p=P)
    with tc.tile_pool(name="wt_sb", bufs=3) as wtp, \
         tc.tile_pool(name="h_ps", bufs=4, space="PSUM") as hp, \
         tc.tile_pool(name="bv_ps", bufs=2, space="PSUM") as bvp, \
         tc.tile_pool(name="bv_sb", bufs=2) as bvs, \
         tc.tile_pool(name="ob_sb", bufs=4) as obp:
        for v in range(NV):
            wt = wtp.tile([P, NK, P], F32R, name="wt")
            nc.sync.dma_start(wt[:], wth_r[:, :, ts(v, P)])
            pbv = bvp.tile([P, 1], F32, name="pbv")
            for k in range(NK):
                nc.tensor.matmul(pbv[:], lhsT=wt[:, k, :].bitcast(F32),
                                 rhs=beta_t[:, k : k + 1],
                                 start=(k == 0), stop=(k == NK - 1))
            bv = bvs.tile([P, 1], F32, name="bv")
            nc.vector.tensor_copy(bv[:], pbv[:])
            for j in range(NJ):
                ph = hp.tile([P, 512], F32, name="ph")
                for k in range(NK):
                    nc.tensor.matmul(ph[:], lhsT=wt[:, k, :],
                                     rhs=yt[:, k, ts(j, 512)],
                                     start=(k == 0), stop=(k == NK - 1))
                ob = obp.tile([P, 512], F32, name="ob")
                if (v + j) % 2 == 0:
                    nc.vector.tensor_scalar_add(ob[:], ph[:], bv[:, 0:1])
                else:
                    nc.scalar.add(ob[:], ph[:], bv[:, 0:1])
                nc.sync.dma_start(out[ts(v, P), ts(j, 512)], ob[:])

    _pp_cm.__exit__(None, None, None)
    _cp_cm.__exit__(None, None, None)


_CACHE = {}


def _build(debug_taps=False):
    key = ("nc", debug_taps)
    if key in _CACHE:
        return _CACHE[key]
    nc = bacc.Bacc("TRN2", target_bir_lowering=False, debug=False,
                   enable_asserts=False, num_devices=N_CORES)
    io = {
        "w_emb": nc.dram_tensor("w_emb", [V, D], F32, kind="ExternalInput").ap(),
        "idxs": nc.dram_tensor("idxs", [P, NC], I32, kind="ExternalInput").ap(),
        "gt": nc.dram_tensor("gt", [D, E], F32, kind="ExternalInput").ap(),
        "win": nc.dram_tensor("win", [D, E * R], F32, kind="ExternalInput").ap(),
        "wbd": nc.dram_tensor("wbd", [P, P], F32, kind="ExternalInput").ap(),
        "wout": nc.dram_tensor("wout", [P, D], F32, kind="ExternalInput").ap(),
        "expt": nc.dram_tensor("expt", [E, P], F32, kind="ExternalInput").ap(),
        "wth": nc.dram_tensor("wth", [D, VSH], F32R, kind="ExternalInput").ap(),
        "gamma": nc.dram_tensor("gamma", [1, D], F32, kind="ExternalInput").ap(),
        "beta": nc.dram_tensor("beta", [P, NK], F32, kind="ExternalInput").ap(),
        "out": nc.dram_tensor("out", [VSH, N], F32, kind="ExternalOutput").ap(),
    }
    if debug_taps:
        io["dbg"] = {
            "xtt": nc.dram_tensor("d_xtt", [D, N], F32, kind="ExternalOutput").ap(),
            "yt": nc.dram_tensor("d_yt", [D, N], F32R, kind="ExternalOutput").ap(),
            "oht": nc.dram_tensor("d_oht", [E, N], F32, kind="ExternalOutput").ap(),
            "mask": nc.dram_tensor("d_mask", [P, N], F32, kind="ExternalOutput").ap(),
            "xin": nc.dram_tensor("d_xin", [P, N], F32, kind="ExternalOutput").ap(),
            "ct": nc.dram_tensor("d_ct", [P, N], F32, kind="ExternalOutput").ap(),
            "mr": nc.dram_tensor("d_mr", [2, N], F32, kind="ExternalOutput").ap(),
        }
    with tile.TileContext(nc) as tc:
        _emit(tc, io)
    nc.compile()
    _CACHE[key] = nc
    return nc


def _host_inputs(idx, W_emb, G, W_in, W_rec, W_out, ln_gamma, ln_beta):
    idx = np.asarray(idx).astype(np.int32)          # [B, T]
    W_emb = np.ascontiguousarray(np.asarray(W_emb, dtype=np.float32))
    G = np.asarray(G, dtype=np.float32)
    W_in = np.asarray(W_in, dtype=np.float32)
    W_rec = np.asarray(W_rec, dtype=np.float32)
    W_out = np.asarray(W_out, dtype=np.float32)
    ln_gamma = np.asarray(ln_gamma, dtype=np.float32)
    ln_beta = np.asarray(ln_beta, dtype=np.float32)

    idxn = idx.T.reshape(-1)                         # n = t*B + b
    idxs = np.ascontiguousarray(idxn.reshape(NC, P).T).astype(np.int32)

    win = np.ascontiguousarray(W_in.transpose(1, 0, 2).reshape(D, E * R))
    wbd = np.zeros((E * R, E * R), dtype=np.float32)
    for e in range(E):
        wbd[R * e : R * e + R, R * e : R * e + R] = W_rec[e]

    common = {
        "w_emb": W_emb,
        "idxs": idxs,
        "gt": np.ascontiguousarray(G.T),
        "win": win,
        "wbd": wbd,
        "wout": np.ascontiguousarray(W_out.reshape(E * R, D)),
        "expt": np.ascontiguousarray(np.repeat(np.eye(E, dtype=np.float32), R, axis=1)),
        "gamma": np.ascontiguousarray(ln_gamma.reshape(1, D)),
        "beta": np.ascontiguousarray(ln_beta.reshape(NK, P).T),
    }
    wt_full = np.zeros((D, VSH * N_CORES), dtype=np.float32)
    wt_full[:, :V] = W_emb.T
    in_maps = []
    for c in range(N_CORES):
        m = dict(common)
        m["wth"] = np.ascontiguousarray(wt_full[:, VSH * c : VSH * (c + 1)])
        in_maps.append(m)
    return in_maps


def run(inputs, trace=False):
    nc = _build()
    in_maps = _host_inputs(**inputs)
    try:
        res = run_bass_kernel_spmd(nc, in_maps, core_ids=list(range(N_CORES)),
                                   trace=trace)
    except ModuleNotFoundError:
        res = run_bass_kernel_spmd(nc, in_maps, core_ids=list(range(N_CORES)),
                                   trace=False)
    shards = [r["out"] for r in res.results]         # each [VSH, N]
    full = np.concatenate(shards, axis=0)[:V]        # [V, N]
    logits = full.reshape(V, T, B).transpose(2, 1, 0)
    return np.ascontiguousarray(logits.astype(np.float32)), res


def kernel(**inputs):
    logits, _ = run(inputs, trace=False)
    return logits
